# revision 1
# baseline (speedup 1.0000x reference)
"""L1 loss (mean |yhat - y|) over (64, 128, 4096) fp32 tensors on 8 TRN2 cores.

v9: data-parallel batch shard; flat per-core [128, 32768] view (the global
sum is permutation-invariant so the zero-copy flat re-chunk is valid).

DMA: every tile's yhat half rides the Sync HWDGE ring and its y half the
Scalar ring, so the two rings stay byte-balanced at every instant (the
per-core HBM rate floor under 8-core contention is much higher with two
active rings, and neither ring is left carrying the stream tail alone).
dma_starts are emitted LEAD tiles ahead of the compute in program order so
an activation waiting on data never delays a DMA issue on the in-order
Scalar sequencer.

Tiles taper deeply (6x4096, 3x2048, 1024, 512, 256, 128, 128): no 4-MiB
tile lands near the stream end, so the serial per-tile compute chase after
the last bytes is ~1.5us. The tapered tiles own dedicated SBUF slots so
their DMAs enqueue without waiting on slot releases.

Compute: DVE does only the subtract (~4.4us per big tile); ScalarE does
abs + per-partition sum in one in-place activation(Abs, accum_out) pass
over the diff tile (HW-validated exact) and the final out-DMA, so the
accumulator never needs cross-engine sync. Host sums the partials in
float64.
"""

import numpy as np

import concourse.bacc as bacc
import concourse.mybir as mybir
import concourse.tile as tile
from concourse.bass_utils import run_bass_kernel_spmd

N_CORES = 8
FULL_SHAPE = (64, 128, 4096)
TOTAL_ELEMS = FULL_SHAPE[0] * FULL_SHAPE[1] * FULL_SHAPE[2]  # 33,554,432

P = 128
ELEMS_PER_CORE = TOTAL_ELEMS // N_CORES   # 4,194,304
F_TOTAL = ELEMS_PER_CORE // P             # 32,768

F_MAIN = [4096] * 6 + [2048] * 3          # share pool slots
F_SMALL = [1024, 512, 256, 128, 128]      # dedicated slots
F_TILES = F_MAIN + F_SMALL
assert sum(F_TILES) == F_TOTAL
N_TILES = len(F_TILES)
N_MAIN = len(F_MAIN)

_nc_cache = []


def _build_nc():
    nc = bacc.Bacc("TRN2", target_bir_lowering=False, debug=False)
    yh = nc.declare_dram_parameter("yh", [P, F_TOTAL], mybir.dt.float32, isOutput=False)
    yy = nc.declare_dram_parameter("yy", [P, F_TOTAL], mybir.dt.float32, isOutput=False)
    out = nc.declare_dram_parameter("out", [P, N_TILES], mybir.dt.float32, isOutput=True)

    offs = []
    o = 0
    for f in F_TILES:
        offs.append(o)
        o += f

    with tile.TileContext(nc) as tc:
        with (
            tc.tile_pool(name="ina", bufs=4) as a_pool,
            tc.tile_pool(name="inb", bufs=4) as b_pool,
            tc.tile_pool(name="diff", bufs=2) as diff_pool,
            tc.tile_pool(name="small", bufs=1) as small_pool,
            tc.tile_pool(name="acc", bufs=1) as acc_pool,
        ):
            acc = acc_pool.tile([P, N_TILES], mybir.dt.float32)
            ats, bts, ds = [], [], []
            for i, f in enumerate(F_TILES):
                if i < N_MAIN:
                    ats.append(
                        a_pool.tile([P, f], mybir.dt.float32, tag="a", name=f"a{i}")
                    )
                    bts.append(
                        b_pool.tile([P, f], mybir.dt.float32, tag="b", name=f"b{i}")
                    )
                else:
                    ats.append(
                        small_pool.tile(
                            [P, f], mybir.dt.float32, tag=f"a{i}", name=f"a{i}"
                        )
                    )
                    bts.append(
                        small_pool.tile(
                            [P, f], mybir.dt.float32, tag=f"b{i}", name=f"b{i}"
                        )
                    )
                ds.append(diff_pool.tile([P, f], mybir.dt.float32, tag="d", name=f"d{i}"))

            def load(i):
                f = F_TILES[i]
                nc.sync.dma_start(ats[i][:], yh[:, offs[i] : offs[i] + f])
                nc.scalar.dma_start(bts[i][:], yy[:, offs[i] : offs[i] + f])

            def compute(i):
                f = F_TILES[i]
                nc.vector.tensor_sub(ds[i][:], ats[i][:], bts[i][:])
                nc.scalar.activation(
                    ds[i][:],
                    ds[i][:],
                    mybir.ActivationFunctionType.Abs,
                    accum_out=acc[:, i : i + 1],
                )

            LEAD = 4
            for i in range(LEAD):
                load(i)
            for i in range(N_TILES):
                if i + LEAD < N_TILES:
                    load(i + LEAD)
                compute(i)
            nc.scalar.dma_start(out[:], acc[:])
    nc.compile()
    return nc


def _get_nc():
    if not _nc_cache:
        _nc_cache.append(_build_nc())
    return _nc_cache[0]


def _shard_inputs(yhat: np.ndarray, y: np.ndarray) -> list[dict[str, np.ndarray]]:
    yh = np.ascontiguousarray(yhat, dtype=np.float32).reshape(N_CORES, P, F_TOTAL)
    yy = np.ascontiguousarray(y, dtype=np.float32).reshape(N_CORES, P, F_TOTAL)
    return [{"yh": yh[c], "yy": yy[c]} for c in range(N_CORES)]


def kernel(yhat: np.ndarray, y: np.ndarray) -> np.ndarray:
    nc = _get_nc()
    in_maps = _shard_inputs(yhat, y)
    res = run_bass_kernel_spmd(nc, in_maps, list(range(N_CORES)))
    total = np.float64(0.0)
    for r in res.results:
        total += r["out"].astype(np.float64).sum()
    return np.asarray(total / TOTAL_ELEMS, dtype=np.float32)



# revision 3
# speedup vs baseline: 1.0458x; 1.0458x over previous
"""L1 loss (mean |yhat - y|) over (64, 128, 4096) fp32 tensors on 8 TRN2 cores.

v11: data-parallel batch shard; flat per-core [128, 32768] view (the global
sum is permutation-invariant so the zero-copy flat re-chunk is valid).

DMA: ALL input loads ride the Sync HWDGE ring (one InstDMACopy is split
across all 16 SDMA engines, so a single ring reaches the full ~435 GB/s
fabric rate; v9's trace showed 425-440 GB/s sustained with two rings, so
ring count is not the limiter). v9 put the y-stream's loads on the Scalar
ring, where a sem-lane-recycling wait scheduled in front of a queued
ACTIVATE head-of-line blocked the Scalar engine for 30 us mid-kernel,
starving the whole pipeline (DMA throughput crashed 429 -> 35 GB/s at
t=70us and the last 7.7 MB took 45 us). With zero DMA issues on compute
engines, sem-lane pacing throttles only the Sync sequencer, which is
harmless: its queue stays ~8 tile-pairs deep.

Compute: DVE does only the subtract (~36 us busy); ScalarE does abs +
per-partition sum in one in-place activation(Abs, accum_out) pass over
the diff tile (HW-validated exact) plus the final out-DMA. Both engines
run well under the ~79 us DMA stream time, so neither paces the kernel.
Host sums the partials in float64.

Tiles taper (6x4096, 3x2048, 1024, 512, 256, 128, 128): no 2-MiB tile
lands near the stream end, so the serial per-tile compute chase after the
last bytes is ~1.5us. The tapered tiles own dedicated SBUF slots so their
DMAs enqueue without waiting on slot releases.

(A DVE-only variant using tensor_tensor_reduce max/min pairs crashes the
device on this runtime stack - NRT_EXEC_UNIT_UNRECOVERABLE - so the
sub+abs split stays.)
"""

import numpy as np

import concourse.bacc as bacc
import concourse.mybir as mybir
import concourse.tile as tile
from concourse.bass_utils import run_bass_kernel_spmd

N_CORES = 8
FULL_SHAPE = (64, 128, 4096)
TOTAL_ELEMS = FULL_SHAPE[0] * FULL_SHAPE[1] * FULL_SHAPE[2]  # 33,554,432

P = 128
ELEMS_PER_CORE = TOTAL_ELEMS // N_CORES   # 4,194,304
F_TOTAL = ELEMS_PER_CORE // P             # 32,768

F_MAIN = [4096] * 6 + [2048] * 3          # share pool slots
F_SMALL = [1024, 512, 256, 128, 128]      # dedicated slots
F_TILES = F_MAIN + F_SMALL
assert sum(F_TILES) == F_TOTAL
N_TILES = len(F_TILES)
N_MAIN = len(F_MAIN)

_nc_cache = []


def _build_nc():
    nc = bacc.Bacc("TRN2", target_bir_lowering=False, debug=False)
    yh = nc.declare_dram_parameter("yh", [P, F_TOTAL], mybir.dt.float32, isOutput=False)
    yy = nc.declare_dram_parameter("yy", [P, F_TOTAL], mybir.dt.float32, isOutput=False)
    out = nc.declare_dram_parameter("out", [P, N_TILES], mybir.dt.float32, isOutput=True)

    offs = []
    o = 0
    for f in F_TILES:
        offs.append(o)
        o += f

    with tile.TileContext(nc) as tc:
        with (
            tc.tile_pool(name="ina", bufs=4) as a_pool,
            tc.tile_pool(name="inb", bufs=4) as b_pool,
            tc.tile_pool(name="diff", bufs=2) as diff_pool,
            tc.tile_pool(name="small", bufs=1) as small_pool,
            tc.tile_pool(name="acc", bufs=1) as acc_pool,
        ):
            acc = acc_pool.tile([P, N_TILES], mybir.dt.float32)
            ats, bts, ds = [], [], []
            for i, f in enumerate(F_TILES):
                if i < N_MAIN:
                    ats.append(
                        a_pool.tile([P, f], mybir.dt.float32, tag="a", name=f"a{i}")
                    )
                    bts.append(
                        b_pool.tile([P, f], mybir.dt.float32, tag="b", name=f"b{i}")
                    )
                else:
                    ats.append(
                        small_pool.tile(
                            [P, f], mybir.dt.float32, tag=f"a{i}", name=f"a{i}"
                        )
                    )
                    bts.append(
                        small_pool.tile(
                            [P, f], mybir.dt.float32, tag=f"b{i}", name=f"b{i}"
                        )
                    )
                ds.append(diff_pool.tile([P, f], mybir.dt.float32, tag="d", name=f"d{i}"))

            def load(i):
                f = F_TILES[i]
                nc.sync.dma_start(ats[i][:], yh[:, offs[i] : offs[i] + f])
                nc.sync.dma_start(bts[i][:], yy[:, offs[i] : offs[i] + f])

            def compute(i):
                nc.vector.tensor_sub(ds[i][:], ats[i][:], bts[i][:])
                nc.scalar.activation(
                    ds[i][:],
                    ds[i][:],
                    mybir.ActivationFunctionType.Abs,
                    accum_out=acc[:, i : i + 1],
                )

            LEAD = 4
            for i in range(LEAD):
                load(i)
            for i in range(N_TILES):
                if i + LEAD < N_TILES:
                    load(i + LEAD)
                compute(i)
            nc.scalar.dma_start(out[:], acc[:])
    nc.compile()
    return nc


def _get_nc():
    if not _nc_cache:
        _nc_cache.append(_build_nc())
    return _nc_cache[0]


def _shard_inputs(yhat: np.ndarray, y: np.ndarray) -> list[dict[str, np.ndarray]]:
    yh = np.ascontiguousarray(yhat, dtype=np.float32).reshape(N_CORES, P, F_TOTAL)
    yy = np.ascontiguousarray(y, dtype=np.float32).reshape(N_CORES, P, F_TOTAL)
    return [{"yh": yh[c], "yy": yy[c]} for c in range(N_CORES)]


def kernel(yhat: np.ndarray, y: np.ndarray) -> np.ndarray:
    nc = _get_nc()
    in_maps = _shard_inputs(yhat, y)
    res = run_bass_kernel_spmd(nc, in_maps, list(range(N_CORES)))
    total = np.float64(0.0)
    for r in res.results:
        total += r["out"].astype(np.float64).sum()
    return np.asarray(total / TOTAL_ELEMS, dtype=np.float32)
